# revision 25
# baseline (speedup 1.0000x reference)
"""Trainium2 Bass kernel for MADE autoregressive sampling (rsample).

Structure exploited (degrees mh = arange(512)%63 + 1, sorted):
  - sorted hidden units split exactly into 4 partition groups of 128
    (degrees 1-15 | 16-31 | 32-47 | 48-63); group boundaries align with
    degree boundaries;
  - every h1/h2 unit is final once z_{deg-1} is known -> computed exactly
    once, at step == its degree;
  - cross-group h1->h2 contributions are batched into full 128x128
    matmuls at the 3 group boundaries (steps 15/31/47) and accumulated
    into per-group SBUF prefix tensors (h2preS), staged per step;
  - all matmul operands are bf16 (1 PE pass vs 2 half-rate passes for
    fp32); PSUM accumulation stays fp32;
  - per-step staged adds (ctx projection, h2 prefix) are folded into the
    L1/L2 matmuls themselves: the stage DMA lands the staged rows in
    extra partition rows of the moving operand and the stacked weights
    [W; I] add them during the same PE pass (w1x has two row slots used
    alternately so stage DMAs keep 2-step lookahead).

Layouts (per core, batch shard BS=1024, halves of 512):
  unit-land  : features on partitions, batch on free dim.  Column j of half h
               is batch row r = h*512 + (j%4)*128 + j//4.
  batch-land : output accumulators / z / eps / outputs keep batch on
               partitions (128) x 4 chunks side-by-side in the free dim.
Compute-engine APs start at partition 0 (verifier rejects unaligned
partition bases); every partition-crossing move is a DMA.
"""

import ml_dtypes
import numpy as np

B, D, CTX, H = 8192, 64, 256, 512
NCORES = 8
BS = B // NCORES   # 1024 rows per core
NH = BS // 2       # 512 per half (PSUM-bank moving-operand max)
MMAX = 9           # max units per degree
KZ = D + 2 * MMAX  # 82: z rows + two ctx-stage slots

BF = ml_dtypes.bfloat16


def _structure():
    m0 = np.arange(1, D + 1)
    mh = (np.arange(H) % (D - 1)) + 1
    M1 = (mh[:, None] >= m0[None, :]).astype(np.float32)   # (H, D)
    M2 = (mh[:, None] >= mh[None, :]).astype(np.float32)   # (H, H)
    mo = np.concatenate([m0, m0])
    Mo = (mo[:, None] > mh[None, :]).astype(np.float32)    # (2D, H)
    perm = np.argsort(mh, kind="stable")
    smh = mh[perm]
    S = np.zeros(D, np.int64)
    E = np.zeros(D, np.int64)
    for i in range(1, D):
        S[i] = np.searchsorted(smh, i, side="left")
        E[i] = np.searchsorted(smh, i, side="right")
    return M1, M2, Mo, perm, S, E


_M1, _M2, _Mo, _PERM, _S, _E = _structure()
assert int(_E[15]) == 128 and int(_E[31]) == 256 and int(_E[47]) == 384
# within-half batch column remap: column j <-> shard row (j%4)*128 + j//4
_J = np.arange(NH)
_RMAP = (_J % 4) * 128 + _J // 4

# push schedule: (source group G, target group g2) emitted after step i.
# G->G+1 must land at the boundary step; farther targets are deferred.
_PUSHES = {15: [(0, 1)], 17: [(0, 2)], 19: [(0, 3)],
           31: [(1, 2)], 33: [(1, 3)],
           47: [(2, 3)]}


def _host_weights(W1, b1, Wc, W2, b2, Wo, bo):
    W1m = (W1 * _M1).T[:, _PERM]                     # (64, 512)
    W2m = ((W2 * _M2).T)[_PERM][:, _PERM]            # (512, 512)
    Wom = ((Wo * _Mo).T)[_PERM, :]                   # (512, 128)
    Wcs = Wc[_PERM]                                  # (512, 256)
    b1s_ = b1[_PERM]
    b2s_ = b2[_PERM]

    # stacked L1 weights [W1m slice; I@slot(i%2)] and L2 diag [W2 diag; I]
    w1x = np.zeros((KZ, D * MMAX), np.float32)
    w2dx = np.zeros((2 * MMAX, D * MMAX), np.float32)
    womp = np.zeros((MMAX, D * 2 * D), np.float32)
    b2p = np.zeros((MMAX, D), np.float32)
    for i in range(1, D):
        s, e = int(_S[i]), int(_E[i])
        m = e - s
        w1x[0:D, i * MMAX:i * MMAX + m] = W1m[:, s:e]
        slot = D + MMAX * (i % 2)
        for k in range(m):
            w1x[slot + k, i * MMAX + k] = 1.0
        w2dx[:m, i * MMAX:i * MMAX + m] = W2m[s:e, s:e]
        if i >= 16:
            # identity rows directly after the m diag rows -> the L2 matmul
            # reads exactly 2m fully-written rows of the hx tile
            for k in range(m):
                w2dx[m + k, i * MMAX + k] = 1.0
        womp[:m, i * 2 * D:(i + 1) * 2 * D] = Wom[s:e, :]
        b2p[:m, i] = b2s_[s:e]

    return {
        "w1x": w1x.astype(BF),
        "w2m": np.ascontiguousarray(W2m).astype(BF),
        "wct": np.ascontiguousarray(Wcs.T).astype(BF),        # (256, 512)
        "w2dx": w2dx.astype(BF),
        "womp": womp.astype(BF),
        "b1s": np.ascontiguousarray(b1s_.reshape(4, 128).T, np.float32),
        "b2p": b2p,
        "b2g": np.ascontiguousarray(b2s_.reshape(4, 128).T, np.float32),
        "bo2": np.ascontiguousarray(bo[None, :], np.float32),  # (1, 128)
        "ones": np.ones((1, 128), np.float32),
        "zzb": np.zeros((KZ, BS), np.float32).astype(BF),
    }


_NC_CACHE = {}


def _build():
    if "nc" in _NC_CACHE:
        return _NC_CACHE["nc"]
    from contextlib import ExitStack

    import concourse.mybir as mybir
    import concourse.tile as tile
    from concourse import bacc

    f32 = mybir.dt.float32
    bf16 = mybir.dt.bfloat16
    AF = mybir.ActivationFunctionType
    OP = mybir.AluOpType

    # Scalar engine uses only Exp/Ln/Relu/Identity/Copy, all in the
    # "natural_log_exp_and_others" ACT table; blank the others so the
    # greedy table-selection pass never inserts mid-kernel table loads.
    import concourse.bacc as bacc_mod
    _orig_tables = bacc_mod.get_activation_tables

    def _one_table(arch):
        tabs = _orig_tables(arch)
        return {k: (v if k == "natural_log_exp_and_others" else set())
                for k, v in tabs.items()}

    bacc_mod.get_activation_tables = _one_table

    nc = bacc.Bacc("TRN2", target_bir_lowering=False)

    ctxT_d = nc.dram_tensor("ctxT", [CTX, BS], bf16, kind="ExternalInput")
    epsB_d = nc.dram_tensor("epsB", [2, 128, 4 * D], f32, kind="ExternalInput")
    w1x_d = nc.dram_tensor("w1x", [KZ, D * MMAX], bf16, kind="ExternalInput")
    w2m_d = nc.dram_tensor("w2m", [H, H], bf16, kind="ExternalInput")
    wct_d = nc.dram_tensor("wct", [CTX, H], bf16, kind="ExternalInput")
    w2dx_d = nc.dram_tensor("w2dx", [2 * MMAX, D * MMAX], bf16,
                            kind="ExternalInput")
    womp_d = nc.dram_tensor("womp", [MMAX, D * 2 * D], bf16,
                            kind="ExternalInput")
    b1s_d = nc.dram_tensor("b1s", [128, 4], f32, kind="ExternalInput")
    b2p_d = nc.dram_tensor("b2p", [MMAX, D], f32, kind="ExternalInput")
    b2g_d = nc.dram_tensor("b2g", [128, 4], f32, kind="ExternalInput")
    bo2_d = nc.dram_tensor("bo2", [1, 2 * D], f32, kind="ExternalInput")
    ones_d = nc.dram_tensor("ones", [1, 128], f32, kind="ExternalInput")
    zzb_d = nc.dram_tensor("zzb", [KZ, BS], bf16, kind="ExternalInput")

    # outputs, batch-major (BS, D); rows r = h*512 + ch*128 + p
    zo_d = nc.dram_tensor("zo", [BS, D], f32, kind="ExternalOutput")
    mo_d = nc.dram_tensor("mo", [BS, D], f32, kind="ExternalOutput")
    so_d = nc.dram_tensor("so", [BS, D], f32, kind="ExternalOutput")

    with tile.TileContext(nc) as tc, ExitStack() as ctx:
        const = ctx.enter_context(tc.tile_pool(name="const", bufs=1))
        work = ctx.enter_context(tc.tile_pool(name="work", bufs=4))
        pout = ctx.enter_context(tc.tile_pool(name="pout", bufs=1, space="PSUM"))
        pscr = ctx.enter_context(tc.tile_pool(name="pscr", bufs=2, space="PSUM"))

        # ---- constants / state ----
        w1x = const.tile([KZ, D * MMAX], bf16)
        nc.sync.dma_start(w1x[:, :], w1x_d[:, :])
        w2m = [const.tile([128, H], bf16, name=f"w2m{g}") for g in range(4)]
        for g in range(4):
            nc.sync.dma_start(w2m[g][:, :], w2m_d[g * 128:(g + 1) * 128, :])
        wct = [const.tile([128, H], bf16, name=f"wct{k}") for k in range(2)]
        for k in range(2):
            nc.sync.dma_start(wct[k][:, :], wct_d[k * 128:(k + 1) * 128, :])
        ctxT = [const.tile([128, BS], bf16, name=f"ctxTs{k}") for k in range(2)]
        for k in range(2):
            nc.sync.dma_start(ctxT[k][:, :], ctxT_d[k * 128:(k + 1) * 128, :])
        w2dx = const.tile([2 * MMAX, D * MMAX], bf16)
        nc.sync.dma_start(w2dx[:, :], w2dx_d[:, :])
        womp = const.tile([MMAX, D * 2 * D], bf16)
        nc.sync.dma_start(womp[:, :], womp_d[:, :])
        b1s = const.tile([128, 4], f32)
        nc.sync.dma_start(b1s[:, :], b1s_d[:, :])
        b2p = const.tile([MMAX, D], f32)
        nc.sync.dma_start(b2p[:, :], b2p_d[:, :])
        b2g = const.tile([128, 4], f32)
        nc.sync.dma_start(b2g[:, :], b2g_d[:, :])
        bo2 = const.tile([1, 2 * D], f32)
        nc.sync.dma_start(bo2[:, :], bo2_d[:, :])
        ones = const.tile([1, 128], f32)
        nc.sync.dma_start(ones[:, :], ones_d[:, :])
        epsB = [const.tile([128, 4 * D], f32, name=f"epsB{h}") for h in range(2)]
        for h in range(2):
            nc.sync.dma_start(epsB[h][:, :], epsB_d[h, :, :])
        # z state (rows 0..63) + two ctx-stage row slots (64..81)
        zTbx = const.tile([KZ, BS], bf16)
        nc.sync.dma_start(zTbx[:, :], zzb_d[:, :])

        h1g = [const.tile([128, BS], bf16, name=f"h1g{g}") for g in range(4)]
        cbg = [const.tile([128, BS], bf16, name=f"cbg{g}") for g in range(4)]
        h2preS = {g: const.tile([128, BS], bf16, name=f"h2preS{g}")
                  for g in (1, 2, 3)}
        h2preF = {g: const.tile([128, BS], f32, name=f"h2preF{g}")
                  for g in (2, 3)}
        muB = [const.tile([128, 4 * D], f32, name=f"muB{h}") for h in range(2)]
        scB = [const.tile([128, 4 * D], f32, name=f"scB{h}") for h in range(2)]
        zB = [const.tile([128, 4 * D], f32, name=f"zB{h}") for h in range(2)]
        zt2 = [const.tile([128, 4 * D], f32, name=f"zt2{h}") for h in range(2)]

        # persistent transposed output accumulators: [batch 128, 4ch x 256]
        # (each chunk at stride 256 so chunks 0,1 sit in one PSUM bank and
        # 2,3 in the next -> consecutive L3 matmuls can alternate banks)
        outp = [pout.tile([128, 4 * 256], f32, name=f"outp{h}") for h in range(2)]

        def ov(h, ch):            # (128, 128) chunk view of the accumulator
            return outp[h][:, ch * 256:ch * 256 + 128]

        def ocol(h, o):           # (128, 4) strided column view, output o
            return outp[h][:, :].rearrange("p (c o) -> p c o", c=4)[:, :, o]

        def bcol(t, i):           # (128, 4) strided column of a (128, 4*D) tile
            return t[:, :].rearrange("p (c d) -> p c d", c=4)[:, :, i]

        # ---- init: bias rows ----
        for h in range(2):
            for ch in range(4):
                nc.tensor.matmul(ov(h, ch), ones[0:1, :], bo2[0:1, :],
                                 start=True, stop=True)

        # ---- init: ctx projection (+b1), unit-land, bf16 out ----
        for g in range(4):
            for h in range(2):
                cs = slice(h * NH, (h + 1) * NH)
                pc = pscr.tile([128, NH], f32, tag="p2", name=f"pc{g}_{h}")
                for k in range(2):
                    nc.tensor.matmul(pc[:, :], wct[k][:, g * 128:(g + 1) * 128],
                                     ctxT[k][:, cs], start=(k == 0), stop=(k == 1))
                nc.scalar.activation(cbg[g][:, cs], pc[:, :], AF.Identity,
                                     bias=b1s[:, g:g + 1])

        # ---- stage helpers ----
        hx = {}       # step -> (2*MMAX, BS) tile: rows 0..8 h1n, 9..17 h2 stage

        def emit_cstage(i):
            # ctx rows for step i into zTbx slot i%2 (after L1 of step i-2
            # released the slot; emission point guarantees program order)
            if i >= D:
                return
            s, e = int(_S[i]), int(_E[i])
            g, r0, m = s // 128, s % 128, e - s
            slot = D + MMAX * (i % 2)
            nc.gpsimd.dma_start(zTbx[slot:slot + m, :], cbg[g][r0:r0 + m, :])

        def alloc_hx(i):
            if i >= D:
                return
            hx[i] = work.tile([2 * MMAX, BS], bf16, tag="hx", name=f"hx{i}")

        def emit_h2stage(i):
            s, e = int(_S[i]), int(_E[i])
            g, r0, m = s // 128, s % 128, e - s
            nc.gpsimd.dma_start(hx[i][m:2 * m, :],
                                h2preS[g][r0:r0 + m, :])

        def extract(i):
            for h in range(2):
                # softplus(x) = ln(exp(x) + 1)
                nc.scalar.activation(bcol(scB[h], i), ocol(h, D + i), AF.Exp)
                nc.scalar.activation(bcol(scB[h], i), bcol(scB[h], i), AF.Ln,
                                     bias=1.0)
                zt = work.tile([128, 4], f32, tag="zt", name=f"zt{i}_{h}")
                nc.vector.tensor_tensor(zt[:, :], bcol(scB[h], i),
                                        bcol(epsB[h], i), OP.mult)
                zr = work.tile([128, 4], bf16, tag=f"zr{h}", name=f"zr{i}_{h}")
                nc.vector.tensor_tensor(zr[:, :], zt[:, :], ocol(h, i), OP.add)
                # scatter z_i into zTbx row i (col j = (j%4)*128 + j//4 ->
                # contiguous 8B runs); separate queues keep the two
                # half-chains independent
                eng = nc.scalar if h == 0 else nc.sync
                eng.dma_start(
                    zTbx[i:i + 1, h * NH:(h + 1) * NH].rearrange(
                        "a (p c) -> a p c", c=4),
                    zr[:, :])

        alloc_hx(1)
        alloc_hx(2)
        emit_cstage(1)
        emit_cstage(2)
        extract(0)

        next_h2 = 16  # first step that consumes an h2preS stage

        for i in range(1, D):
            s, e = int(_S[i]), int(_E[i])
            g, r0, m = s // 128, s % 128, e - s
            csl = [slice(h * NH, (h + 1) * NH) for h in range(2)]

            # L2 within-group prefix (independent of z_i -> runs early)
            p2t = []
            for h in range(2):
                p2 = pscr.tile([MMAX, NH], f32, tag="p2", name=f"p2_{i}_{h}")
                p2t.append(p2)
                if r0 > 0:
                    nc.tensor.matmul(p2[0:m, :], w2m[g][0:r0, s:e],
                                     h1g[g][0:r0, csl[h]],
                                     start=True, stop=False)
            # L1: W1 z + staged ctx rows in one pass via stacked [W1; I]
            p1t = []
            for h in range(2):
                p1 = pscr.tile([MMAX, NH], f32, tag="p1", name=f"p1_{i}_{h}")
                p1t.append(p1)
                nc.tensor.matmul(p1[0:m, :], w1x[:, i * MMAX:i * MMAX + m],
                                 zTbx[:, csl[h]], start=True, stop=True)
            # relu engine split chosen so every engine queue receives its
            # step ops in dependency-arrival order (no head-of-line blocks):
            # scalar: r1(h0), r1(h1), exp/ln; vector: r2(h0), r2(h1), TTs
            nc.scalar.activation(hx[i][0:m, 0:NH], p1t[0][0:m, :], AF.Relu)
            nc.scalar.activation(hx[i][0:m, NH:BS], p1t[1][0:m, :], AF.Relu)
            for h in range(2):
                nc.sync.dma_start(h1g[g][r0:r0 + m, csl[h]],
                                  hx[i][0:m, csl[h]])
            # L2 diag + staged h2-prefix rows in one pass via [W2d; I]
            k2 = 2 * m if g >= 1 else m

            h2n = []
            for h in range(2):
                h2n.append(work.tile([MMAX, NH], bf16, tag=f"h2n{h}",
                                     name=f"h2n_{i}_{h}"))

            def l2_and_l3(h):
                nc.tensor.matmul(p2t[h][0:m, :],
                                 w2dx[0:k2, i * MMAX:i * MMAX + m],
                                 hx[i][0:k2, csl[h]],
                                 start=(r0 == 0), stop=True)
                t = h2n[h]
                if g == 0:
                    nc.vector.tensor_scalar(t[0:m, :], p2t[h][0:m, :],
                                            b2p[0:m, i:i + 1], 0.0,
                                            OP.add, OP.max)
                else:
                    nc.vector.tensor_relu(t[0:m, :], p2t[h][0:m, :])
                h2v = t[0:m, :].rearrange("m (b c) -> m b c", c=4)
                for ch in (0, 2, 1, 3):
                    nc.tensor.matmul(ov(h, ch), h2v[:, :, ch],
                                     womp[0:m, i * 2 * D:(i + 1) * 2 * D],
                                     start=False, stop=True)

            # serve half 0's chain fully before half 1's L2/L3 so the PE
            # queue never parks half 0's work behind half 1's late deps
            l2_and_l3(0)
            l2_and_l3(1)
            extract(i)

            # group-boundary pushes: h1 group G complete -> batch its
            # contribution to future groups with full 128x128 matmuls.
            # Only G->G+1 is needed immediately; pushes to later groups are
            # deferred to the following (quieter) steps.
            for G, g2 in _PUSHES.get(i, ()):
                for h in range(2):
                    cs = csl[h]
                    pp = pscr.tile([128, NH], f32, tag="p2",
                                   name=f"push{G}_{g2}_{h}")
                    nc.tensor.matmul(pp[:, :],
                                     w2m[G][:, g2 * 128:(g2 + 1) * 128],
                                     h1g[G][:, cs], start=True, stop=True)
                    if G == 0:
                        tgt = h2preS[1] if g2 == 1 else h2preF[g2]
                        nc.scalar.activation(tgt[:, cs], pp[:, :], AF.Identity,
                                             bias=b2g[:, g2:g2 + 1])
                    else:
                        nc.vector.tensor_tensor(h2preF[g2][:, cs],
                                                h2preF[g2][:, cs],
                                                pp[:, :], OP.add)
                        if g2 == G + 1:
                            nc.scalar.activation(h2preS[g2][:, cs],
                                                 h2preF[g2][:, cs],
                                                 AF.Identity)

            # stage emission for step i+2 (program order: after this step's
            # L1 released the zTbx slot / after pushes filled h2preS)
            alloc_hx(i + 2)
            emit_cstage(i + 2)
            while (next_h2 < D and next_h2 - 2 <= i
                   and i >= 16 * (int(_S[next_h2]) // 128) - 1):
                emit_h2stage(next_h2)
                next_h2 += 1

        # ---- bulk extraction of mu and z (scales accumulated per step) ----
        for h in range(2):
            mu_src = outp[h][:, :].rearrange("p (c o) -> p c o", c=4)[:, :, 0:D]
            mu_dst = muB[h][:, :].rearrange("p (c d) -> p c d", c=4)[:, :, :]
            nc.scalar.activation(mu_dst, mu_src, AF.Identity)
            nc.vector.tensor_tensor(zt2[h][:, :], scB[h][:, :], epsB[h][:, :],
                                    OP.mult)
            nc.vector.tensor_tensor(zB[h][:, :], zt2[h][:, :], muB[h][:, :],
                                    OP.add)

        # ---- outputs (batch-major rows r = h*512 + ch*128 + p) ----
        for h in range(2):
            dst = slice(h * NH, (h + 1) * NH)
            for name_d, t in ((zo_d, zB[h]), (mo_d, muB[h]), (so_d, scB[h])):
                nc.sync.dma_start(
                    name_d[dst, :].rearrange("(c p) d -> p c d", c=4),
                    t[:, :].rearrange("p (c d) -> p c d", c=4))

    nc.compile()
    _NC_CACHE["nc"] = nc
    return nc


def kernel(context, eps, W1, b1, Wc, W2, b2, Wo, bo, _trace=False):
    from concourse.bass_utils import run_bass_kernel_spmd

    context = np.asarray(context, np.float32)
    eps = np.asarray(eps, np.float32)
    wd = _host_weights(np.asarray(W1, np.float32), np.asarray(b1, np.float32),
                       np.asarray(Wc, np.float32), np.asarray(W2, np.float32),
                       np.asarray(b2, np.float32), np.asarray(Wo, np.float32),
                       np.asarray(bo, np.float32))

    in_maps = []
    cols = np.concatenate([_RMAP, NH + _RMAP])
    for c in range(NCORES):
        sl = slice(c * BS, (c + 1) * BS)
        ctx_s = context[sl]                       # (1024, 256)
        eps_s = eps[sl]                           # (1024, 64)
        im = dict(wd)
        im["ctxT"] = np.ascontiguousarray(ctx_s[cols].T).astype(BF)
        im["epsB"] = np.ascontiguousarray(
            eps_s.reshape(2, 4, 128, D).transpose(0, 2, 1, 3).reshape(
                2, 128, 4 * D))
        in_maps.append(im)

    nc = _build()
    res = run_bass_kernel_spmd(nc, in_maps, core_ids=list(range(NCORES)),
                               trace=_trace)
    z = np.concatenate([r["zo"] for r in res.results], axis=0)
    mus = np.concatenate([r["mo"] for r in res.results], axis=0)
    scales = np.concatenate([r["so"] for r in res.results], axis=0)
    if _trace:
        kernel.last_exec_time_ns = res.exec_time_ns
        kernel.last_results = res
    return z, mus, scales
